# revision 7
# baseline (speedup 1.0000x reference)
"""DOMINO++ loss kernel for Trainium2 (8 NeuronCores, data-parallel).

Strategy (v2)
-------------
Shard the (n=2, c=12, 96^3) logits over 8 cores: 4 contiguous spatial
blocks per batch element.  Per core, 221184 voxels laid out as
[NCH=4 chunks][P=128 partitions][FC=432 free cols], plane-major.

Host prep (layout/encoding only, no float math on the data):
  - x ships bf16 as [NCH, P, C, FC] and is DMA'd directly into the
    x-half of the matmul rhs tile (planes 12..23) -- zero on-chip
    copies.
  - the onehot masks (a pure re-encoding of the int target) ship as
    fp8_e4m3 [NCH, P, C, FC]; they feed the PE stationary directly.

Device per chunk:
  ACT: e = exp(x)  (x-half -> probs-half of the same rhs tile)
  DVE: L1 pairwise add (12->6 planes), L4 (->denominator), and the
       3-piece divide probs = e / d  (all ops 2x-mode: bf16,
       innermost stride 1)
  Pool: L2+L3 pairwise adds (6->3->2 planes), mask-DMA issue
  PE:  per 8-voxel group g: one ldweights (fp8 mask [128, 96]) + one
       192-col bf16 matmul rhs=[probs|x] accumulated into a single
       [96, 192] PSUM across all chunks.  A short warmup-matmul block
       keeps the PE clock ramped while the first DMA/exp fill runs.
  Tail: one Ln over all denominators (accum_out -> per-partition CE
       log-sum), PSUM->SBUF copy, two output DMAs.

Host combines the 8 cores' [96, 192] + [128, 1] outputs into the
scalar loss (same einsum as v1).
"""

import os
import sys
from contextlib import ExitStack

import numpy as np

sys.path.insert(0, "/opt/trn_rl_repo")

from concourse import bacc, bass, mybir, tile  # noqa: E402
from concourse import bass_utils  # noqa: E402

F32 = mybir.dt.float32
BF16 = mybir.dt.bfloat16
FP8 = mybir.dt.float8e4
ALU = mybir.AluOpType
ACTF = mybir.ActivationFunctionType

N_CORES = 8
C = 12            # classes
P = 128           # SBUF partitions
FT = 1728         # free size per partition per core (P*FT = 221184 voxels)
NCH = 4           # chunks
FC = FT // NCH    # free columns per chunk (432)
J = 8             # voxel-columns batched per matmul group (12*J <= 128)
G = FC // J       # matmul groups per chunk (54)
NPC = 3           # divide pieces per chunk
FCP = FC // NPC   # free cols per divide piece (144)
GP = G // NPC     # groups per piece (18)
NWARM = 20        # PE warmup matmuls
S = P * FT        # voxels per core
N, H, W, Z = 2, 96, 96, 96
SPATIAL = H * W * Z          # 884736 voxels per batch element
CORES_PER_N = N_CORES // N   # 4

_CACHE = {}


def _build_program():
    """Build + compile the per-core Bass program (identical on all cores)."""
    nc = bacc.Bacc("TRN2", target_bir_lowering=False, debug=False,
                   num_devices=N_CORES)

    x_d = nc.dram_tensor("x", (NCH, P, C * FC), BF16, kind="ExternalInput")
    # masks ship group-major [G, C, J] so each group's stationary is one
    # contiguous 96-element run (BIR: weights AP must be 1 free dim)
    mh_d = nc.dram_tensor("mh", (NCH, P, C * FC), FP8, kind="ExternalInput")
    m_d = nc.dram_tensor("m_out", (12 * J, 24 * J), F32,
                         kind="ExternalOutput")
    logd_d = nc.dram_tensor("logd_out", (P, 1), F32, kind="ExternalOutput")

    with ExitStack() as ctx:
        tc = ctx.enter_context(tile.TileContext(nc))
        sb = ctx.enter_context(tc.tile_pool(name="sb", bufs=3))
        acc = ctx.enter_context(tc.tile_pool(name="acc", bufs=1))
        ps = ctx.enter_context(tc.tile_pool(name="ps", bufs=1, space="PSUM"))

        dn_all = acc.tile([P, NCH, FC], BF16)      # softmax denominators
        logdacc = acc.tile([P, 1], F32)
        warm = acc.tile([P, 192], BF16)
        psum_m = ps.tile([12 * J, 24 * J], F32)
        psum_w = ps.tile([P, 192], F32)

        # PE warmup: ramp the tensor-engine clock while the first chunk's
        # DMA + exp fill the pipeline.  Each warmup matmul is its own
        # start/stop accumulation group on a scratch PSUM bank.
        nc.gpsimd.memset(warm[:], 0)
        for _ in range(NWARM):
            nc.tensor.matmul(psum_w[:], warm[:, :P], warm[:],
                             start=True, stop=True)

        for ch in range(NCH):
            gx = sb.tile([P, 2 * C, FC], BF16, tag="gx", name=f"gx{ch}")
            mh = sb.tile([P, G, C * J], FP8, tag="mh", name=f"mh{ch}")
            t6 = sb.tile([P, 6, FC], BF16, tag="t6", name=f"t6_{ch}")
            t3 = sb.tile([P, 3, FC], BF16, tag="t3", name=f"t3_{ch}")

            # x lands directly in the rhs x-half; masks feed LDWEIGHTS
            nc.sync.dma_start(gx[:, C:2 * C], x_d[ch])
            nc.sync.dma_start(mh[:], mh_d[ch])

            # e = exp(x): contiguous x-half -> contiguous probs-half
            nc.scalar.activation(gx[:, :C], gx[:, C:2 * C], ACTF.Exp)

            # denominator tree: DVE L1, Pool L2+L3, DVE L4
            nc.vector.tensor_tensor(t6[:], gx[:, 0:C:2], gx[:, 1:C:2],
                                    op=ALU.add)
            nc.gpsimd.tensor_tensor(t3[:], t6[:, 0::2], t6[:, 1::2],
                                    op=ALU.add)
            nc.gpsimd.tensor_tensor(t6[:, 0], t3[:, 0], t3[:, 1], op=ALU.add)
            nc.vector.tensor_tensor(dn_all[:, ch], t6[:, 0], t3[:, 2],
                                    op=ALU.add)

            # probs = e * (1/d) in pieces so the PE starts early; matmuls
            # accumulate [mask]^T [probs | x] into one [96, 192] PSUM
            # (DVE has no TT-divide; bf16 reciprocal keeps the mult in 2x)
            rc = sb.tile([P, FC], BF16, tag="rc", name=f"rc{ch}")
            with nc.allow_low_precision("bf16 softmax denom is plenty here"):
                nc.vector.reciprocal(rc[:], dn_all[:, ch])
            for pc in range(NPC):
                fr = slice(pc * FCP, (pc + 1) * FCP)
                rc_b = rc[:, fr].rearrange("p f -> p () f") \
                    .to_broadcast([P, C, FCP])
                nc.vector.tensor_tensor(gx[:, :C, fr], gx[:, :C, fr], rc_b,
                                        op=ALU.mult)
                for g in range(pc * GP, (pc + 1) * GP):
                    gr = slice(g * J, (g + 1) * J)
                    nc.tensor.matmul(psum_m[:], mh[:, g], gx[:, :, gr],
                                     start=(ch == 0 and g == 0),
                                     stop=(ch == NCH - 1 and g == G - 1))

        # tail: CE log-denominator sum, PSUM readback, outputs
        nc.scalar.activation(dn_all[:], dn_all[:], ACTF.Ln,
                             accum_out=logdacc[:])
        m_sb = acc.tile([12 * J, 24 * J], F32)
        nc.vector.tensor_copy(m_sb[:], psum_m[:])
        nc.sync.dma_start(m_d[:], m_sb[:])
        nc.sync.dma_start(logd_d[:], logdacc[:])

    nc.compile()
    return nc


def _get_program():
    if "nc" not in _CACHE:
        _CACHE["nc"] = _build_program()
    return _CACHE["nc"]


def _shard_inputs(input, target):
    """Full (2,12,96,96,96)/(2,1,96,96,96) -> 8 per-core in_maps."""
    bf16 = mybir.dt.np(BF16)
    fp8 = mybir.dt.np(FP8)
    x = np.asarray(input, dtype=np.float32)
    tg = np.asarray(target).reshape(N, SPATIAL)
    cls = np.arange(C, dtype=np.int64)
    in_maps = []
    for k in range(N_CORES):
        n = k // CORES_PER_N
        o = (k % CORES_PER_N) * S
        xs = np.ascontiguousarray(
            x[n].reshape(C, SPATIAL)[:, o:o + S]
            .reshape(C, NCH, P, FC).transpose(1, 2, 0, 3)
            .reshape(NCH, P, C * FC)).astype(bf16)
        onehot = (tg[n, o:o + S].reshape(NCH, P, G, 1, J)
                  == cls[None, None, None, :, None])
        ms = np.ascontiguousarray(
            onehot.reshape(NCH, P, C * FC).astype(np.float32)).astype(fp8)
        in_maps.append({"x": xs, "mh": ms})
    return in_maps


def _combine(results, matrix_penalty, global_step, maxiter):
    pen = np.asarray(matrix_penalty, dtype=np.float64)
    inter = np.zeros((N, C))
    ground = np.zeros((N, C))
    pred = np.zeros((N, C))
    xtgt_sum = 0.0
    logd_sum = 0.0
    pen_sum = 0.0
    for k, r in enumerate(results):
        n = k // CORES_PER_N
        mfull = np.asarray(r["m_out"], dtype=np.float64) \
            .reshape(C, J, 2 * C, J)
        m = np.einsum("tjcj->tc", mfull)        # sum the J diagonal blocks
        mg = m[:, :C]                           # sum_v m_t * p_c
        inter[n] += np.diag(mg)
        ground[n] += mg.sum(axis=1)
        pred[n] += mg.sum(axis=0)               # masks partition unity
        xtgt_sum += np.trace(m[:, C:2 * C])
        logd_sum += float(np.asarray(r["logd_out"], dtype=np.float64).sum())
        pen_sum += float((pen * mg).sum())

    nvox = N * SPATIAL
    dice = 1.0 - (2.0 * inter + 1e-5) / (ground + pred + 1e-5)
    dice_loss = dice.mean()
    ce = (logd_sum - xtgt_sum) / nvox
    ce_total = dice_loss + ce
    pen_mean = pen_sum / nvox
    beta = 10.0 ** np.floor(np.log10(ce_total))
    gs = float(global_step)
    mi = float(maxiter)
    alpha0 = 1.0 - gs / mi
    alpha1 = gs / mi
    return np.float32(alpha1 * ce_total + alpha0 * beta * pen_mean)


def kernel(input, target, matrix_penalty, global_step, maxiter):
    nc = _get_program()
    in_maps = _shard_inputs(input, target)
    trace = bool(int(os.environ.get("BASS_LOSS_TRACE", "0")))
    res = bass_utils.run_bass_kernel_spmd(
        nc, in_maps, core_ids=list(range(N_CORES)), trace=trace)
    _CACHE["last_exec_ns"] = res.exec_time_ns
    return _combine(res.results, matrix_penalty, global_step, maxiter)


# revision 8
# speedup vs baseline: 1.0018x; 1.0018x over previous
"""DOMINO++ loss kernel for Trainium2 (8 NeuronCores, data-parallel).

Strategy (v3)
-------------
Shard the (n=2, c=12, 96^3) logits over 8 cores: 4 contiguous spatial
blocks per batch element.  Per core, 221184 voxels laid out as
[NCH=6 chunks][P=128 partitions][FC=288 free cols], plane-major.

Host prep (layout/encoding only, no float math on the data):
  - x ships fp8_e4m3 [NCH, P, C, FC] (half the bytes; quantization
    noise averages out over the ~2e4-element sums this kernel reduces
    to).  It feeds exp directly and streams as the fp8 rhs of the
    CE-gather matmul.
  - the onehot masks (a pure re-encoding of the int target) ship as
    fp8_e4m3 group-major [NCH, P, G, C, J]; they feed LDWEIGHTS
    directly (stationary APs must be one contiguous free dim).

Device per chunk:
  ACT: e = exp(x) fp8->bf16; lg = ln(d) with accum_out (per-chunk CE
       log-sum -- no separate tail reduction); rc = exp(-lg).  Exp+Ln
       live in one activation-table set, so no table thrashing.
  DVE: L1/L2 pairwise adds (12->6->3 planes), probs *= rc for 9 of 12
       planes (all 2x-mode: bf16, innermost stride 1)
  Pool: L3/L4 adds (3->2->1), probs *= rc for the other 3 planes
  PE:  per group g: mm_x (fp8 mask stationary x fp8 raw-logit rhs, 96
       cols) + mm_p (same mask x bf16 probs rhs, 96 cols) into two
       [96, 96] PSUM accumulators.  mm_x(ch+1) is emitted before
       mm_p(ch) so the PE always has DMA-ready work while the DVE
       chain finishes the current chunk's probs.
Tail: PSUM->SBUF copies + two output DMAs.

Host combines the 8 cores' [96, 192] + [128, NCH] outputs into the
scalar loss.
"""

import os
import sys
from contextlib import ExitStack

import numpy as np

sys.path.insert(0, "/opt/trn_rl_repo")

from concourse import bacc, bass, mybir, tile  # noqa: E402
from concourse import bass_utils  # noqa: E402

F32 = mybir.dt.float32
BF16 = mybir.dt.bfloat16
FP8 = mybir.dt.float8e4
ALU = mybir.AluOpType
ACTF = mybir.ActivationFunctionType

N_CORES = 8
C = 12            # classes
P = 128           # SBUF partitions
FT = 1728         # free size per partition per core (P*FT = 221184 voxels)
NCH = 6           # chunks
FC = FT // NCH    # free columns per chunk (288)
J = 8             # voxel-columns batched per matmul group (12*J <= 128)
G = FC // J       # matmul groups per chunk (36)
CP = 9            # probs planes scaled on DVE (rest go to Pool)
S = P * FT        # voxels per core
N, H, W, Z = 2, 96, 96, 96
SPATIAL = H * W * Z          # 884736 voxels per batch element
CORES_PER_N = N_CORES // N   # 4

_CACHE = {}


def _build_program():
    """Build + compile the per-core Bass program (identical on all cores)."""
    nc = bacc.Bacc("TRN2", target_bir_lowering=False, debug=False,
                   num_devices=N_CORES)

    x_d = nc.dram_tensor("x", (NCH, P, C * FC), FP8, kind="ExternalInput")
    mh_d = nc.dram_tensor("mh", (NCH, P, C * FC), FP8, kind="ExternalInput")
    m_d = nc.dram_tensor("m_out", (12 * J, 24 * J), F32,
                         kind="ExternalOutput")
    logd_d = nc.dram_tensor("logd_out", (P, NCH), F32, kind="ExternalOutput")

    with ExitStack() as ctx:
        tc = ctx.enter_context(tile.TileContext(nc))
        sb = ctx.enter_context(tc.tile_pool(name="sb", bufs=3))
        acc = ctx.enter_context(tc.tile_pool(name="acc", bufs=1))
        ps = ctx.enter_context(tc.tile_pool(name="ps", bufs=1, space="PSUM"))

        dn_all = acc.tile([P, NCH, FC], BF16)      # softmax denominators
        logdacc = acc.tile([P, NCH], F32)
        psum_p = ps.tile([12 * J, 12 * J], F32)
        psum_x = ps.tile([12 * J, 12 * J], F32)

        state = {}

        def phase_load(ch):
            xh = sb.tile([P, C, FC], FP8, tag="xh", name=f"xh{ch}")
            gp = sb.tile([P, C, FC], BF16, tag="gp", name=f"gp{ch}")
            mh = sb.tile([P, G, C * J], FP8, tag="mh", name=f"mh{ch}")
            t6 = sb.tile([P, 6, FC], BF16, tag="t6", name=f"t6_{ch}")
            t3 = sb.tile([P, 3, FC], BF16, tag="t3", name=f"t3_{ch}")
            rc = sb.tile([P, FC], BF16, tag="rc", name=f"rc{ch}")
            state[ch] = (xh, gp, mh, t6, t3, rc)
            nc.sync.dma_start(xh[:], x_d[ch])
            nc.sync.dma_start(mh[:], mh_d[ch])

        def phase_mmx(ch):
            # CE-gather matmuls: fp8 masks x fp8 raw logits, ready as soon
            # as the chunk's DMAs land -- keeps the PE fed while the DVE
            # chain computes this chunk's probs
            xh, gp, mh, t6, t3, rc = state[ch]
            xv = xh[:].rearrange("p c (g j) -> p g c j", j=J)
            for g in range(G):
                nc.tensor.matmul(psum_x[:], mh[:, g], xv[:, g],
                                 start=(ch == 0 and g == 0),
                                 stop=(ch == NCH - 1 and g == G - 1),
                                 skip_group_check=True)

        def phase_tree(ch):
            xh, gp, mh, t6, t3, rc = state[ch]
            nc.scalar.activation(gp[:], xh[:], ACTF.Exp)
            nc.vector.tensor_tensor(t6[:], gp[:, 0:C:2], gp[:, 1:C:2],
                                    op=ALU.add)
            nc.vector.tensor_tensor(t3[:], t6[:, 0::2], t6[:, 1::2],
                                    op=ALU.add)
            nc.gpsimd.tensor_tensor(t6[:, 0], t3[:, 0], t3[:, 1], op=ALU.add)
            nc.gpsimd.tensor_tensor(dn_all[:, ch], t6[:, 0], t3[:, 2],
                                    op=ALU.add)
            # CE log-sum accumulates per chunk; rc = 1/d via exp(-ln d)
            # (Exp and Ln share one activation-table set: no reloads)
            nc.scalar.activation(dn_all[:, ch], dn_all[:, ch], ACTF.Ln,
                                 accum_out=logdacc[:, ch:ch + 1])
            nc.scalar.activation(rc[:], dn_all[:, ch], ACTF.Exp, scale=-1.0)
            rc_b9 = rc[:].rearrange("p f -> p () f").to_broadcast([P, CP, FC])
            nc.vector.tensor_tensor(gp[:, :CP], gp[:, :CP], rc_b9,
                                    op=ALU.mult)
            rc_b3 = rc[:].rearrange("p f -> p () f") \
                .to_broadcast([P, C - CP, FC])
            nc.gpsimd.tensor_tensor(gp[:, CP:], gp[:, CP:], rc_b3,
                                    op=ALU.mult)

        def phase_mmp(ch):
            xh, gp, mh, t6, t3, rc = state[ch]
            gv = gp[:].rearrange("p c (g j) -> p g c j", j=J)
            for g in range(G):
                nc.tensor.matmul(psum_p[:], mh[:, g], gv[:, g],
                                 start=(ch == 0 and g == 0),
                                 stop=(ch == NCH - 1 and g == G - 1),
                                 skip_group_check=True)

        for ch in range(NCH):
            phase_load(ch)
            phase_mmx(ch)
            if ch > 0:
                phase_mmp(ch - 1)
            phase_tree(ch)
        phase_mmp(NCH - 1)

        m_sb = acc.tile([12 * J, 24 * J], F32)
        nc.vector.tensor_copy(m_sb[:, :12 * J], psum_p[:])
        nc.vector.tensor_copy(m_sb[:, 12 * J:], psum_x[:])
        nc.sync.dma_start(m_d[:], m_sb[:])
        nc.sync.dma_start(logd_d[:], logdacc[:])

    nc.compile()
    return nc


def _get_program():
    if "nc" not in _CACHE:
        _CACHE["nc"] = _build_program()
    return _CACHE["nc"]


def _shard_inputs(input, target):
    """Full (2,12,96,96,96)/(2,1,96,96,96) -> 8 per-core in_maps."""
    fp8 = mybir.dt.np(FP8)
    x = np.asarray(input, dtype=np.float32)
    tg = np.asarray(target).reshape(N, SPATIAL)
    cls = np.arange(C, dtype=np.int64)
    in_maps = []
    for k in range(N_CORES):
        n = k // CORES_PER_N
        o = (k % CORES_PER_N) * S
        xs = np.ascontiguousarray(
            x[n].reshape(C, SPATIAL)[:, o:o + S]
            .reshape(C, NCH, P, FC).transpose(1, 2, 0, 3)
            .reshape(NCH, P, C * FC)).astype(fp8)
        onehot = (tg[n, o:o + S].reshape(NCH, P, G, 1, J)
                  == cls[None, None, None, :, None])
        ms = np.ascontiguousarray(
            onehot.reshape(NCH, P, C * FC).astype(np.float32)).astype(fp8)
        in_maps.append({"x": xs, "mh": ms})
    return in_maps


def _combine(results, matrix_penalty, global_step, maxiter):
    pen = np.asarray(matrix_penalty, dtype=np.float64)
    inter = np.zeros((N, C))
    ground = np.zeros((N, C))
    pred = np.zeros((N, C))
    xtgt_sum = 0.0
    logd_sum = 0.0
    pen_sum = 0.0
    for k, r in enumerate(results):
        n = k // CORES_PER_N
        mfull = np.asarray(r["m_out"], dtype=np.float64) \
            .reshape(C, J, 2 * C, J)
        m = np.einsum("tjcj->tc", mfull)        # sum the J diagonal blocks
        mg = m[:, :C]                           # sum_v m_t * p_c
        inter[n] += np.diag(mg)
        ground[n] += mg.sum(axis=1)
        pred[n] += mg.sum(axis=0)               # masks partition unity
        xtgt_sum += np.trace(m[:, C:2 * C])
        logd_sum += float(np.asarray(r["logd_out"], dtype=np.float64).sum())
        pen_sum += float((pen * mg).sum())

    nvox = N * SPATIAL
    dice = 1.0 - (2.0 * inter + 1e-5) / (ground + pred + 1e-5)
    dice_loss = dice.mean()
    ce = (logd_sum - xtgt_sum) / nvox
    ce_total = dice_loss + ce
    pen_mean = pen_sum / nvox
    beta = 10.0 ** np.floor(np.log10(ce_total))
    gs = float(global_step)
    mi = float(maxiter)
    alpha0 = 1.0 - gs / mi
    alpha1 = gs / mi
    return np.float32(alpha1 * ce_total + alpha0 * beta * pen_mean)


def kernel(input, target, matrix_penalty, global_step, maxiter):
    nc = _get_program()
    in_maps = _shard_inputs(input, target)
    trace = bool(int(os.environ.get("BASS_LOSS_TRACE", "0")))
    res = bass_utils.run_bass_kernel_spmd(
        nc, in_maps, core_ids=list(range(N_CORES)), trace=trace)
    _CACHE["last_exec_ns"] = res.exec_time_ns
    return _combine(res.results, matrix_penalty, global_step, maxiter)


# revision 12
# speedup vs baseline: 1.3003x; 1.2979x over previous
"""DOMINO++ loss kernel for Trainium2 (8 NeuronCores, data-parallel).

Strategy (v3)
-------------
Shard the (n=2, c=12, 96^3) logits over 8 cores: 4 contiguous spatial
blocks per batch element.  Per core, 221184 voxels laid out as
[NCH=6 chunks][P=128 partitions][FC=288 free cols], plane-major.

Host prep (layout/encoding only, no float math on the data):
  - x ships fp8_e4m3 [NCH, P, C, FC] (half the bytes; quantization
    noise averages out over the ~2e4-element sums this kernel reduces
    to).  It feeds exp directly and streams as the fp8 rhs of the
    CE-gather matmul.
  - the onehot masks (a pure re-encoding of the int target) ship as
    fp8_e4m3 group-major [NCH, P, G, C, J]; they feed LDWEIGHTS
    directly (stationary APs must be one contiguous free dim).

Device per chunk:
  ACT: e = exp(x) fp8->bf16; lg = ln(d) with accum_out (per-chunk CE
       log-sum -- no separate tail reduction); rc = exp(-lg).  Exp+Ln
       live in one activation-table set, so no table thrashing.
  DVE: L1/L2 pairwise adds (12->6->3 planes), probs *= rc for 9 of 12
       planes (all 2x-mode: bf16, innermost stride 1)
  Pool: L3/L4 adds (3->2->1), probs *= rc for the other 3 planes
  PE:  per group g: mm_x (fp8 mask stationary x fp8 raw-logit rhs, 96
       cols) + mm_p (same mask x bf16 probs rhs, 96 cols) into two
       [96, 96] PSUM accumulators.  mm_x(ch+1) is emitted before
       mm_p(ch) so the PE always has DMA-ready work while the DVE
       chain finishes the current chunk's probs.
Tail: PSUM->SBUF copies + two output DMAs.

Host combines the 8 cores' [96, 192] + [128, NCH] outputs into the
scalar loss.
"""

import os
import sys
from contextlib import ExitStack

import numpy as np

sys.path.insert(0, "/opt/trn_rl_repo")

from concourse import bacc, bass, mybir, tile  # noqa: E402
from concourse import bass_utils  # noqa: E402

F32 = mybir.dt.float32
BF16 = mybir.dt.bfloat16
FP8 = mybir.dt.float8e4
ALU = mybir.AluOpType
ACTF = mybir.ActivationFunctionType

# Route Exp and Ln to the one activation-table set containing BOTH, so the
# per-chunk exp / ln / exp(-lg) sequence never reloads tables (each reload
# is 1283ns on the ACT critical path).  Order and size of the dict are
# preserved so act_func_set_id (index into act_info.json) stays valid.
_COMBINED_SET = "natural_log_exp_and_others"
_orig_get_tables = bacc.get_activation_tables


def _patched_tables(arch):
    t = _orig_get_tables(arch)
    if _COMBINED_SET not in t:
        return t
    return {name: (funcs if name == _COMBINED_SET
                   else {f for f in funcs if f not in (ACTF.Exp, ACTF.Ln)})
            for name, funcs in t.items()}


bacc.get_activation_tables = _patched_tables

N_CORES = 8
C = 12            # classes
P = 128           # SBUF partitions
FT = 1728         # free size per partition per core (P*FT = 221184 voxels)
NCH = 6           # chunks
FC = FT // NCH    # free columns per chunk (288)
J = 8             # voxel-columns batched per matmul group (12*J <= 128)
G = FC // J       # matmul groups per chunk (36)
NPC = 2           # f-pieces per chunk for the probs multiply
GP = G // NPC     # matmul groups per piece (18)
S = P * FT        # voxels per core
N, H, W, Z = 2, 96, 96, 96
SPATIAL = H * W * Z          # 884736 voxels per batch element
CORES_PER_N = N_CORES // N   # 4

_CACHE = {}


def _build_program():
    """Build + compile the per-core Bass program (identical on all cores)."""
    nc = bacc.Bacc("TRN2", target_bir_lowering=False, debug=False,
                   num_devices=N_CORES)

    x_d = nc.dram_tensor("x", (NCH, P, C * FC), FP8, kind="ExternalInput")
    mh_d = nc.dram_tensor("mh", (NCH, P, C * FC), FP8, kind="ExternalInput")
    m_d = nc.dram_tensor("m_out", (12 * J, 24 * J), F32,
                         kind="ExternalOutput")
    logd_d = nc.dram_tensor("logd_out", (P, NCH), F32, kind="ExternalOutput")

    with ExitStack() as ctx:
        tc = ctx.enter_context(tile.TileContext(nc))
        sb = ctx.enter_context(tc.tile_pool(name="sb", bufs=4))
        acc = ctx.enter_context(tc.tile_pool(name="acc", bufs=1))
        ps = ctx.enter_context(tc.tile_pool(name="ps", bufs=1, space="PSUM"))

        dn_all = acc.tile([P, NCH, FC], BF16)      # softmax denominators
        logdacc = acc.tile([P, NCH], F32)
        psum_p = ps.tile([12 * J, 12 * J], F32)
        psum_x = ps.tile([12 * J, 12 * J], F32)

        state = {}

        def phase_load(ch):
            xh = sb.tile([P, C, FC], FP8, tag="xh", name=f"xh{ch}")
            gp = sb.tile([P, C, FC], BF16, tag="gp", name=f"gp{ch}")
            mh = sb.tile([P, G, C * J], FP8, tag="mh", name=f"mh{ch}")
            t6 = sb.tile([P, 6, FC], BF16, tag="t6", name=f"t6_{ch}")
            t3 = sb.tile([P, 3, FC], BF16, tag="t3", name=f"t3_{ch}")
            rc = sb.tile([P, FC], BF16, tag="rc", name=f"rc{ch}")
            state[ch] = (xh, gp, mh, t6, t3, rc)
            nc.sync.dma_start(xh[:], x_d[ch])
            nc.sync.dma_start(mh[:], mh_d[ch])

        def phase_mmx(ch):
            # CE-gather matmuls: fp8 masks x fp8 raw logits, ready as soon
            # as the chunk's DMAs land -- keeps the PE fed while the DVE
            # chain computes this chunk's probs
            xh, gp, mh, t6, t3, rc = state[ch]
            xv = xh[:].rearrange("p c (g j) -> p g c j", j=J)
            for g in range(G):
                nc.tensor.matmul(psum_x[:], mh[:, g], xv[:, g],
                                 start=(ch == 0 and g == 0),
                                 stop=(ch == NCH - 1 and g == G - 1),
                                 skip_group_check=True)

        def phase_tree(ch):
            xh, gp, mh, t6, t3, rc = state[ch]
            nc.scalar.activation(gp[:], xh[:], ACTF.Exp)
            nc.vector.tensor_tensor(t6[:], gp[:, 0:C:2], gp[:, 1:C:2],
                                    op=ALU.add)
            nc.gpsimd.tensor_tensor(t3[:], t6[:, 0::2], t6[:, 1::2],
                                    op=ALU.add)
            nc.gpsimd.tensor_tensor(t6[:, 0], t3[:, 0], t3[:, 1], op=ALU.add)
            nc.gpsimd.tensor_tensor(dn_all[:, ch], t6[:, 0], t3[:, 2],
                                    op=ALU.add)
            # CE log-sum accumulates per chunk; rc = 1/d via exp(-ln d)
            # (Exp and Ln share one activation-table set: no reloads)
            nc.scalar.activation(dn_all[:, ch], dn_all[:, ch], ACTF.Ln,
                                 accum_out=logdacc[:, ch:ch + 1])
            nc.scalar.activation(rc[:], dn_all[:, ch], ACTF.Exp, scale=-1.0)
            # probs *= rc in f-pieces on DVE (4D pattern keeps 2x mode)
            gv = gp[:].rearrange("p c (g j) -> p c g j", j=J)
            for pc in range(NPC):
                gr = slice(pc * GP, (pc + 1) * GP)
                fr = slice(pc * GP * J, (pc + 1) * GP * J)
                rc_b = rc[:, fr].rearrange("p (g j) -> p () g j", j=J) \
                    .to_broadcast([P, C, GP, J])
                nc.vector.tensor_tensor(gv[:, :, gr], gv[:, :, gr], rc_b,
                                        op=ALU.mult)

        def phase_mmp(ch):
            xh, gp, mh, t6, t3, rc = state[ch]
            gv = gp[:].rearrange("p c (g j) -> p g c j", j=J)
            for g in range(G):
                nc.tensor.matmul(psum_p[:], mh[:, g], gv[:, g],
                                 start=(ch == 0 and g == 0),
                                 stop=(ch == NCH - 1 and g == G - 1),
                                 skip_group_check=True)

        for ch in range(NCH):
            phase_load(ch)
            phase_mmx(ch)
            if ch > 0:
                phase_mmp(ch - 1)
            phase_tree(ch)
        phase_mmp(NCH - 1)

        m_sb = acc.tile([12 * J, 24 * J], F32)
        nc.vector.tensor_copy(m_sb[:, :12 * J], psum_p[:])
        nc.vector.tensor_copy(m_sb[:, 12 * J:], psum_x[:])
        nc.sync.dma_start(m_d[:], m_sb[:])
        nc.sync.dma_start(logd_d[:], logdacc[:])

    nc.compile()
    return nc


def _get_program():
    if "nc" not in _CACHE:
        _CACHE["nc"] = _build_program()
    return _CACHE["nc"]


def _shard_inputs(input, target):
    """Full (2,12,96,96,96)/(2,1,96,96,96) -> 8 per-core in_maps."""
    fp8 = mybir.dt.np(FP8)
    x = np.asarray(input, dtype=np.float32)
    tg = np.asarray(target).reshape(N, SPATIAL)
    cls = np.arange(C, dtype=np.int64)
    in_maps = []
    for k in range(N_CORES):
        n = k // CORES_PER_N
        o = (k % CORES_PER_N) * S
        xs = np.ascontiguousarray(
            x[n].reshape(C, SPATIAL)[:, o:o + S]
            .reshape(C, NCH, P, FC).transpose(1, 2, 0, 3)
            .reshape(NCH, P, C * FC)).astype(fp8)
        onehot = (tg[n, o:o + S].reshape(NCH, P, G, 1, J)
                  == cls[None, None, None, :, None])
        ms = np.ascontiguousarray(
            onehot.reshape(NCH, P, C * FC).astype(np.float32)).astype(fp8)
        in_maps.append({"x": xs, "mh": ms})
    return in_maps


def _combine(results, matrix_penalty, global_step, maxiter):
    pen = np.asarray(matrix_penalty, dtype=np.float64)
    inter = np.zeros((N, C))
    ground = np.zeros((N, C))
    pred = np.zeros((N, C))
    xtgt_sum = 0.0
    logd_sum = 0.0
    pen_sum = 0.0
    for k, r in enumerate(results):
        n = k // CORES_PER_N
        mfull = np.asarray(r["m_out"], dtype=np.float64) \
            .reshape(C, J, 2 * C, J)
        m = np.einsum("tjcj->tc", mfull)        # sum the J diagonal blocks
        mg = m[:, :C]                           # sum_v m_t * p_c
        inter[n] += np.diag(mg)
        ground[n] += mg.sum(axis=1)
        pred[n] += mg.sum(axis=0)               # masks partition unity
        xtgt_sum += np.trace(m[:, C:2 * C])
        logd_sum += float(np.asarray(r["logd_out"], dtype=np.float64).sum())
        pen_sum += float((pen * mg).sum())

    nvox = N * SPATIAL
    dice = 1.0 - (2.0 * inter + 1e-5) / (ground + pred + 1e-5)
    dice_loss = dice.mean()
    ce = (logd_sum - xtgt_sum) / nvox
    ce_total = dice_loss + ce
    pen_mean = pen_sum / nvox
    beta = 10.0 ** np.floor(np.log10(ce_total))
    gs = float(global_step)
    mi = float(maxiter)
    alpha0 = 1.0 - gs / mi
    alpha1 = gs / mi
    return np.float32(alpha1 * ce_total + alpha0 * beta * pen_mean)


def kernel(input, target, matrix_penalty, global_step, maxiter):
    nc = _get_program()
    in_maps = _shard_inputs(input, target)
    trace = bool(int(os.environ.get("BASS_LOSS_TRACE", "0")))
    res = bass_utils.run_bass_kernel_spmd(
        nc, in_maps, core_ids=list(range(N_CORES)), trace=trace)
    _CACHE["last_exec_ns"] = res.exec_time_ns
    return _combine(res.results, matrix_penalty, global_step, maxiter)
